# revision 4
# baseline (speedup 1.0000x reference)
"""Batch-assign-probability (VQ codebook softmax) kernel for 8 Trainium2 cores.

Math: for each valid row x (D=512), over K=256 centers c_k:
    softmax_k(-||x - c_k||^2) == softmax_k(2 x.c_k - ||c_k||^2)
(the ||x||^2 term is constant over k and cancels in softmax).

Sharding: batch B=16 split across 8 cores (2 batches = 2048 valid rows per
core); the small centers table is replicated. Host prep: slice the valid
(unmasked) timesteps, transpose x to [D, rows] so the contraction dim lands
on SBUF partitions, and fold the constant terms into `ct = (2*centers)^T`
and `negc2 = -||c||^2` (bias folded into the matmul via a ones-row matmul).

Device (per core, Tile framework):
  - load ct [512,256] + negc2 once; stream x^T in 512KB groups (256 rows)
  - per 128-row tile: 4 fp32 matmuls (d-chunks) + 1 bias matmul -> PSUM
    logits [128,256]; reduce_max(negate) -> ACT exp(bias=-max, accum sum)
    -> reciprocal -> scale; DMA out 256-row groups.
"""

import numpy as np

import concourse.bacc as bacc
import concourse.tile as tile
from concourse import mybir
from concourse.bass_utils import run_bass_kernel_spmd

B, T, W, C, K = 16, 2048, 512, 1, 256
VALID_T = 1024
D = W * C                       # 512
N_CORES = 8
B_PER_CORE = B // N_CORES       # 2
ROWS = B_PER_CORE * VALID_T     # 2048 rows per core
P = 128
D_CHUNKS = D // P               # 4
GROUP_ROWS = 256                # rows per x DMA (512 KB) / out DMA (256 KB)
N_GROUPS = ROWS // GROUP_ROWS   # 8
SUBTILES = GROUP_ROWS // P      # 2

# Matmul input dtype: float32 (exact, 4 cyc/row) or float32r (1 cyc/row at
# free dim >= 256, reduced precision on HW).
MM_DT = mybir.dt.float32

_CACHE: dict = {}


def _build_bass():
    f32 = mybir.dt.float32
    nc = bacc.Bacc()
    xT = nc.declare_dram_parameter("xT", [D, ROWS], f32, isOutput=False)
    ct = nc.declare_dram_parameter("ct", [D, K], f32, isOutput=False)
    negc2 = nc.declare_dram_parameter("negc2", [1, K], f32, isOutput=False)
    out = nc.declare_dram_parameter("out", [ROWS, K], f32, isOutput=True)

    # DRAM views (einops-style on access patterns)
    xT_v = xT.rearrange("(c p) (g r) -> g p c r", p=P, r=GROUP_ROWS)  # [8,128,4,256]
    ct_v = ct.rearrange("(c p) k -> p c k", p=P)                      # [128,4,256]
    out_v = out.rearrange("(g s r) k -> g r s k", s=SUBTILES, r=P)    # [8,128,2,256]

    with tile.TileContext(nc) as tc:
        with (
            tc.tile_pool(name="singles", bufs=1) as singles,
            tc.tile_pool(name="xpool", bufs=3) as xpool,
            tc.tile_pool(name="opool", bufs=3) as opool,
            tc.tile_pool(name="small", bufs=8) as small,
            tc.tile_pool(name="psum", bufs=4, space="PSUM") as psum,
        ):
            ct_sb = singles.tile([P, D_CHUNKS, K], f32)
            nc.sync.dma_start(out=ct_sb[:], in_=ct_v)
            negc2_sb = singles.tile([1, K], f32)
            nc.sync.dma_start(out=negc2_sb[:], in_=negc2[:])
            ones_sb = singles.tile([1, P], f32)
            nc.vector.memset(ones_sb[:], 1.0)

            for g in range(N_GROUPS):
                xg = xpool.tile([P, D_CHUNKS, GROUP_ROWS], f32)
                nc.sync.dma_start(out=xg[:], in_=xT_v[g])
                og = opool.tile([P, SUBTILES, K], f32)
                for s in range(SUBTILES):
                    ps = psum.tile([P, K], f32)
                    for c in range(D_CHUNKS):
                        nc.tensor.matmul(
                            ps[:],
                            lhsT=xg[:, c, s * P:(s + 1) * P].bitcast(MM_DT),
                            rhs=ct_sb[:, c, :].bitcast(MM_DT),
                            start=(c == 0),
                            stop=False,
                        )
                    nc.tensor.matmul(
                        ps[:],
                        lhsT=ones_sb[:, :].bitcast(MM_DT),
                        rhs=negc2_sb[:, :].bitcast(MM_DT),
                        start=False,
                        stop=True,
                    )
                    negm = small.tile([P, 1], f32)
                    nc.vector.reduce_max(
                        out=negm[:], in_=ps[:], axis=mybir.AxisListType.X, negate=True
                    )
                    esum = small.tile([P, 1], f32)
                    nc.scalar.activation(
                        out=og[:, s, :],
                        in_=ps[:],
                        func=mybir.ActivationFunctionType.Exp,
                        bias=negm[:],
                        scale=1.0,
                        accum_out=esum[:],
                    )
                    rinv = small.tile([P, 1], f32)
                    nc.vector.reciprocal(out=rinv[:], in_=esum[:])
                    nc.vector.tensor_scalar_mul(og[:, s, :], og[:, s, :], rinv[:])
                nc.sync.dma_start(out=out_v[g], in_=og[:])
    nc.finalize()
    return nc


def get_nc():
    if "nc" not in _CACHE:
        _CACHE["nc"] = _build_bass()
    return _CACHE["nc"]


def prep_inputs(y_pred: np.ndarray, mask: np.ndarray, centers: np.ndarray):
    """Host-side shard prep: valid-timestep slice, per-core transpose,
    center-table folding. Returns in_maps for cores 0..7."""
    x = np.ascontiguousarray(y_pred.reshape(B, T, D))
    masktime = np.asarray(mask).reshape(B, T, D)[0, :, 0]
    valid_idx = np.nonzero(masktime == 0)[0][:VALID_T]
    assert valid_idx.shape[0] == VALID_T
    # contiguous fast path (setup uses first VALID_T timesteps)
    if valid_idx[0] == 0 and valid_idx[-1] == VALID_T - 1:
        xv = x[:, :VALID_T]                    # [B, VALID_T, D]
    else:
        xv = x[:, valid_idx]
    centers = np.asarray(centers, dtype=np.float32)
    ct = np.ascontiguousarray((2.0 * centers).T)                    # [D, K]
    negc2 = -(centers * centers).sum(axis=1, dtype=np.float32)      # [K]
    negc2 = np.ascontiguousarray(negc2.reshape(1, K), dtype=np.float32)
    in_maps = []
    for core in range(N_CORES):
        xc = xv[core * B_PER_CORE:(core + 1) * B_PER_CORE].reshape(ROWS, D)
        xTc = np.ascontiguousarray(xc.T)                            # [D, ROWS]
        in_maps.append({"xT": xTc, "ct": ct, "negc2": negc2})
    return in_maps


def kernel(y_pred: np.ndarray, mask: np.ndarray, centers: np.ndarray,
           **run_kwargs) -> np.ndarray:
    in_maps = prep_inputs(y_pred, mask, centers)
    nc = get_nc()
    res = run_bass_kernel_spmd(nc, in_maps, core_ids=list(range(N_CORES)),
                               **run_kwargs)
    _CACHE["last_results"] = res
    out = np.concatenate(
        [r["out"].reshape(B_PER_CORE, VALID_T, K) for r in res.results], axis=0
    )
    return out.astype(np.float32, copy=False)


# revision 12
# speedup vs baseline: 1.2437x; 1.2437x over previous
"""Batch-assign-probability (VQ codebook softmax) kernel for 8 Trainium2 cores.

Math: for each valid row x (D=512), over K=256 centers c_k:
    softmax_k(-||x - c_k||^2) == softmax_k(2 x.c_k - ||c_k||^2)
(the ||x||^2 term is constant over k and cancels in softmax).

Sharding: batch B=16 split across 8 cores (2 batches = 2048 valid rows per
core); the small centers table is replicated. Host prep: slice the valid
(unmasked) timesteps, transpose x to [D, rows] so the contraction dim lands
on SBUF partitions, fold the 2x scale into ct = (2*centers)^T, and split
x / ct into bf16 hi+lo pairs for a 3-pass full-rate matmul:
    x.ct ~= xh.cth + xh.ctl + xl.cth        (error ~1e-4 relative)
The -||c||^2 bias is folded in as one contraction-dim-3 matmul against a
3-level bf16 split of the bias (ones rows on the x side).

Device (per core, Tile framework):
  - load ct hi/lo + bias once; stream x hi/lo in 1MiB groups (512 rows)
  - per 128-row tile: 12 bf16 matmuls (4 d-chunks x 3 passes) + 1 bias
    matmul -> PSUM logits [128,256]; reduce_max(negate) -> ACT
    exp(bias=-max, accum sum) -> reciprocal -> scale; 512KB out DMAs.
"""

import numpy as np
import ml_dtypes

import concourse.bacc as bacc
import concourse.tile as tile
from concourse import mybir
from concourse.bass_utils import run_bass_kernel_spmd

B, T, W, C, K = 16, 2048, 512, 1, 256
VALID_T = 1024
D = W * C                       # 512
N_CORES = 8
B_PER_CORE = B // N_CORES       # 2
ROWS = B_PER_CORE * VALID_T     # 2048 rows per core
P = 128
D_CHUNKS = D // P               # 4
GROUP_ROWS = 512                # rows per x DMA (1 MiB hi+lo) / out DMA (512 KB)
N_GROUPS = ROWS // GROUP_ROWS   # 4
SUBTILES = GROUP_ROWS // P      # 4

BF16_NP = ml_dtypes.bfloat16

_CACHE: dict = {}


def _build_bass():
    f32 = mybir.dt.float32
    bf16 = mybir.dt.bfloat16
    nc = bacc.Bacc()
    # x hi/lo packed: [2, D, ROWS] bf16
    xhl = nc.declare_dram_parameter("xhl", [2, D, ROWS], bf16, isOutput=False)
    cthl = nc.declare_dram_parameter("cthl", [2, D, K], bf16, isOutput=False)
    bias3 = nc.declare_dram_parameter("bias3", [3, K], bf16, isOutput=False)
    ones3 = nc.declare_dram_parameter("ones3", [3, P], bf16, isOutput=False)
    out = nc.declare_dram_parameter("out", [ROWS, K], f32, isOutput=True)

    # DRAM views (einops-style on access patterns)
    x_v = xhl.rearrange("h (c p) (g r) -> g p h c r", p=P, r=GROUP_ROWS)
    ct_v = cthl.rearrange("h (c p) k -> p h c k", p=P)          # [128,2,4,256]
    out_v = out.rearrange("(g s r) k -> g r s k", s=SUBTILES, r=P)

    with tile.TileContext(nc) as tc:
        with (
            tc.tile_pool(name="singles", bufs=1) as singles,
            tc.tile_pool(name="xpool", bufs=3) as xpool,
            tc.tile_pool(name="opool", bufs=3) as opool,
            tc.tile_pool(name="small", bufs=8) as small,
            tc.tile_pool(name="psum", bufs=8, space="PSUM") as psum,
        ):
            ct_sb = singles.tile([P, 2, D_CHUNKS, K], bf16)
            nc.sync.dma_start(out=ct_sb[:], in_=ct_v)
            bias_sb = singles.tile([3, K], bf16)
            nc.sync.dma_start(out=bias_sb[:], in_=bias3[:])
            ones_sb = singles.tile([3, P], bf16)
            nc.sync.dma_start(out=ones_sb[:], in_=ones3[:])

            for g in range(N_GROUPS):
                xg = xpool.tile([P, 2, D_CHUNKS, GROUP_ROWS], bf16)
                nc.sync.dma_start(out=xg[:], in_=x_v[g])
                og = opool.tile([P, SUBTILES, K], f32)
                for s in range(SUBTILES):
                    rsl = slice(s * P, (s + 1) * P)
                    ps = psum.tile([P, K], f32)
                    first = True
                    for c in range(D_CHUNKS):
                        for xh_i, ct_i in ((0, 0), (0, 1), (1, 0)):
                            nc.tensor.matmul(
                                ps[:],
                                lhsT=xg[:, xh_i, c, rsl],
                                rhs=ct_sb[:, ct_i, c, :],
                                start=first,
                                stop=False,
                            )
                            first = False
                    nc.tensor.matmul(
                        ps[:],
                        lhsT=ones_sb[:, :],
                        rhs=bias_sb[:, :],
                        start=False,
                        stop=True,
                    )
                    negm = small.tile([P, 1], f32)
                    nc.vector.reduce_max(
                        out=negm[:], in_=ps[:], axis=mybir.AxisListType.X, negate=True
                    )
                    esum = small.tile([P, 1], f32)
                    nc.scalar.activation(
                        out=og[:, s, :],
                        in_=ps[:],
                        func=mybir.ActivationFunctionType.Exp,
                        bias=negm[:],
                        scale=1.0,
                        accum_out=esum[:],
                    )
                    rinv = small.tile([P, 1], f32)
                    nc.vector.reciprocal(out=rinv[:], in_=esum[:])
                    nc.vector.tensor_scalar_mul(og[:, s, :], og[:, s, :], rinv[:])
                nc.sync.dma_start(out=out_v[g], in_=og[:])
    nc.finalize()
    return nc


def get_nc():
    if "nc" not in _CACHE:
        _CACHE["nc"] = _build_bass()
    return _CACHE["nc"]


def _split_hi_lo(a: np.ndarray) -> np.ndarray:
    """[*shape] f32 -> [2, *shape] bf16 with a ~= hi + lo."""
    hi = a.astype(BF16_NP)
    lo = (a - hi.astype(np.float32)).astype(BF16_NP)
    return np.stack([hi, lo])


def prep_inputs(y_pred: np.ndarray, mask: np.ndarray, centers: np.ndarray):
    """Host-side shard prep: valid-timestep slice, per-core transpose,
    bf16 hi/lo splits, center-table folding. Returns in_maps for cores 0..7."""
    x = np.ascontiguousarray(y_pred.reshape(B, T, D))
    masktime = np.asarray(mask).reshape(B, T, D)[0, :, 0]
    valid_idx = np.nonzero(masktime == 0)[0][:VALID_T]
    assert valid_idx.shape[0] == VALID_T
    # contiguous fast path (setup uses first VALID_T timesteps)
    if valid_idx[0] == 0 and valid_idx[-1] == VALID_T - 1:
        xv = x[:, :VALID_T]                    # [B, VALID_T, D]
    else:
        xv = x[:, valid_idx]
    centers = np.asarray(centers, dtype=np.float32)
    cthl = np.ascontiguousarray(_split_hi_lo((2.0 * centers).T))    # [2, D, K]
    negc2 = -(centers.astype(np.float64) ** 2).sum(axis=1)          # [K]
    b1 = negc2.astype(BF16_NP)
    r1 = negc2 - b1.astype(np.float64)
    b2 = r1.astype(BF16_NP)
    b3 = (r1 - b2.astype(np.float64)).astype(BF16_NP)
    bias3 = np.ascontiguousarray(np.stack([b1, b2, b3]))            # [3, K]
    ones3 = np.ones((3, P), dtype=BF16_NP)
    in_maps = []
    for core in range(N_CORES):
        xc = xv[core * B_PER_CORE:(core + 1) * B_PER_CORE].reshape(ROWS, D)
        xTc = np.ascontiguousarray(xc.T)                            # [D, ROWS]
        in_maps.append({
            "xhl": np.ascontiguousarray(_split_hi_lo(xTc)),
            "cthl": cthl,
            "bias3": bias3,
            "ones3": ones3,
        })
    return in_maps


def kernel(y_pred: np.ndarray, mask: np.ndarray, centers: np.ndarray,
           **run_kwargs) -> np.ndarray:
    in_maps = prep_inputs(y_pred, mask, centers)
    nc = get_nc()
    res = run_bass_kernel_spmd(nc, in_maps, core_ids=list(range(N_CORES)),
                               **run_kwargs)
    _CACHE["last_results"] = res
    out = np.concatenate(
        [r["out"].reshape(B_PER_CORE, VALID_T, K) for r in res.results], axis=0
    )
    return out.astype(np.float32, copy=False)


# revision 13
# speedup vs baseline: 1.3162x; 1.0583x over previous
"""Batch-assign-probability (VQ codebook softmax) kernel for 8 Trainium2 cores.

Math: for each valid row x (D=512), over K=256 centers c_k:
    softmax_k(-||x - c_k||^2) == softmax_k(2 x.c_k - ||c_k||^2)
(the ||x||^2 term is constant over k and cancels in softmax).

Sharding: batch B=16 split across 8 cores (2 batches = 2048 valid rows per
core); the small centers table is replicated. Host prep: slice the valid
(unmasked) timesteps, transpose x to [D, rows] so the contraction dim lands
on SBUF partitions, fold the 2x scale into ct = (2*centers)^T, and split
x / ct into bf16 hi+lo pairs for a 3-pass full-rate matmul:
    x.ct ~= xh.cth + xh.ctl + xl.cth        (error ~2e-4 relative)
The -||c||^2 bias is folded in as one contraction-dim-3 matmul against a
3-level bf16 split of the bias (ones rows on the x side). Host packs each
DMA's source region fully contiguous (8KB-per-partition runs).

Device (per core, Tile framework):
  - load ct hi/lo + bias once; stream x hi/lo in row-groups (small first
    group so the PE starts early, small last group so the exposed softmax
    tail is short); per 128-row tile: 12 bf16 matmuls + 1 bias matmul ->
    PSUM logits [128,256]; reduce_max(negate) -> ACT exp(bias=-max,
    accum sum) -> reciprocal -> scale -> group out DMA.
"""

import numpy as np
import ml_dtypes

import concourse.bacc as bacc
import concourse.tile as tile
from concourse import mybir
from concourse.bass_utils import run_bass_kernel_spmd

B, T, W, C, K = 16, 2048, 512, 1, 256
VALID_T = 1024
D = W * C                       # 512
N_CORES = 8
B_PER_CORE = B // N_CORES       # 2
ROWS = B_PER_CORE * VALID_T     # 2048 rows per core
P = 128
D_CHUNKS = D // P               # 4
GROUPS = [256, 512, 512, 512, 256]   # rows per x/out DMA group
assert sum(GROUPS) == ROWS
X_TOTAL = P * 2 * D_CHUNKS * ROWS    # flat bf16 element count of x param

BF16_NP = ml_dtypes.bfloat16

_CACHE: dict = {}


def _build_bass():
    f32 = mybir.dt.float32
    bf16 = mybir.dt.bfloat16
    nc = bacc.Bacc()
    # x hi/lo, group-major, fully contiguous per group: for each group g
    # (R rows), block [128p, 2h, 4c, R] flattened.
    xp = nc.declare_dram_parameter("xp", [X_TOTAL], bf16, isOutput=False)
    # ct hi block then lo block, each [128p, 4c, 256k] contiguous.
    ctp = nc.declare_dram_parameter("ctp", [2 * P * D_CHUNKS * K], bf16,
                                    isOutput=False)
    bias3 = nc.declare_dram_parameter("bias3", [3, K], bf16, isOutput=False)
    ones3 = nc.declare_dram_parameter("ones3", [3, P], bf16, isOutput=False)
    out = nc.declare_dram_parameter("out", [ROWS, K], f32, isOutput=True)

    out_v = out.rearrange("(t p) k -> p t k", p=P)       # [128, 16, 256]
    ct_half = P * D_CHUNKS * K

    with tile.TileContext(nc) as tc:
        with (
            tc.tile_pool(name="singles", bufs=1) as singles,
            tc.tile_pool(name="xpool", bufs=len(GROUPS)) as xpool,
            tc.tile_pool(name="opool", bufs=3) as opool,
            tc.tile_pool(name="small", bufs=8) as small,
            tc.tile_pool(name="psum", bufs=8, space="PSUM") as psum,
        ):
            ct_sb = singles.tile([P, 2, D_CHUNKS, K], bf16)
            for h in range(2):
                nc.sync.dma_start(
                    out=ct_sb[:, h],
                    in_=ctp[h * ct_half:(h + 1) * ct_half].rearrange(
                        "(p c k) -> p c k", p=P, c=D_CHUNKS),
                )

            # x group DMAs up front — everything stays resident in SBUF
            xgs = []
            xoff = 0
            for g, R in enumerate(GROUPS):
                xg = xpool.tile([P, 2, D_CHUNKS, R], bf16, tag=f"xg{g}")
                n = P * 2 * D_CHUNKS * R
                nc.sync.dma_start(
                    out=xg[:],
                    in_=xp[xoff:xoff + n].rearrange(
                        "(p h c r) -> p h c r", p=P, h=2, c=D_CHUNKS),
                )
                xgs.append(xg)
                xoff += n

            bias_sb = singles.tile([3, K], bf16)
            nc.sync.dma_start(out=bias_sb[:], in_=bias3[:])
            ones_sb = singles.tile([3, P], bf16)
            nc.sync.dma_start(out=ones_sb[:], in_=ones3[:])

            t0 = 0  # running 128-row tile index
            for g, R in enumerate(GROUPS):
                xg = xgs[g]
                subtiles = R // P
                og = opool.tile([P, subtiles, K], f32, tag="og")
                for s in range(subtiles):
                    rsl = slice(s * P, (s + 1) * P)
                    ps = psum.tile([P, K], f32)
                    first = True
                    for c in range(D_CHUNKS):
                        for xh_i, ct_i in ((0, 0), (0, 1), (1, 0)):
                            nc.tensor.matmul(
                                ps[:],
                                lhsT=xg[:, xh_i, c, rsl],
                                rhs=ct_sb[:, ct_i, c, :],
                                start=first,
                                stop=False,
                            )
                            first = False
                    nc.tensor.matmul(
                        ps[:],
                        lhsT=ones_sb[:, :],
                        rhs=bias_sb[:, :],
                        start=False,
                        stop=True,
                    )
                    negm = small.tile([P, 1], f32)
                    nc.vector.reduce_max(
                        out=negm[:], in_=ps[:], axis=mybir.AxisListType.X, negate=True
                    )
                    esum = small.tile([P, 1], f32)
                    nc.scalar.activation(
                        out=og[:, s, :],
                        in_=ps[:],
                        func=mybir.ActivationFunctionType.Exp,
                        bias=negm[:],
                        scale=1.0,
                        accum_out=esum[:],
                    )
                    rinv = small.tile([P, 1], f32)
                    nc.vector.reciprocal(out=rinv[:], in_=esum[:])
                    nc.vector.tensor_scalar_mul(og[:, s, :], og[:, s, :], rinv[:])
                nc.sync.dma_start(out=out_v[:, t0:t0 + subtiles, :], in_=og[:])
                t0 += subtiles
    nc.finalize()
    return nc


def get_nc():
    if "nc" not in _CACHE:
        _CACHE["nc"] = _build_bass()
    return _CACHE["nc"]


def _split_hi_lo(a: np.ndarray) -> tuple[np.ndarray, np.ndarray]:
    hi = a.astype(BF16_NP)
    lo = (a - hi.astype(np.float32)).astype(BF16_NP)
    return hi, lo


def prep_inputs(y_pred: np.ndarray, mask: np.ndarray, centers: np.ndarray):
    """Host-side shard prep: valid-timestep slice, per-core transpose,
    bf16 hi/lo splits, contiguous per-DMA packing."""
    x = np.ascontiguousarray(y_pred.reshape(B, T, D))
    masktime = np.asarray(mask).reshape(B, T, D)[0, :, 0]
    valid_idx = np.nonzero(masktime == 0)[0][:VALID_T]
    assert valid_idx.shape[0] == VALID_T
    if valid_idx[0] == 0 and valid_idx[-1] == VALID_T - 1:
        xv = x[:, :VALID_T]                    # [B, VALID_T, D]
    else:
        xv = x[:, valid_idx]

    centers = np.asarray(centers, dtype=np.float32)
    cth, ctl = _split_hi_lo((2.0 * centers).T)              # [D, K] each
    # [h, c, p, k] -> [h, p, c, k] contiguous
    ct_blocks = [
        np.ascontiguousarray(h.reshape(D_CHUNKS, P, K).transpose(1, 0, 2)).ravel()
        for h in (cth, ctl)
    ]
    ctp = np.ascontiguousarray(np.concatenate(ct_blocks))

    negc2 = -(centers.astype(np.float64) ** 2).sum(axis=1)  # [K]
    b1 = negc2.astype(BF16_NP)
    r1 = negc2 - b1.astype(np.float64)
    b2 = r1.astype(BF16_NP)
    b3 = (r1 - b2.astype(np.float64)).astype(BF16_NP)
    bias3 = np.ascontiguousarray(np.stack([b1, b2, b3]))    # [3, K]
    ones3 = np.ones((3, P), dtype=BF16_NP)

    in_maps = []
    for core in range(N_CORES):
        xc = xv[core * B_PER_CORE:(core + 1) * B_PER_CORE].reshape(ROWS, D)
        xTc = np.ascontiguousarray(xc.T)                    # [D, ROWS]
        xh, xl = _split_hi_lo(xTc)
        # [h, c, p, row] -> [p, h, c, row]
        base = np.stack([xh, xl]).reshape(2, D_CHUNKS, P, ROWS).transpose(2, 0, 1, 3)
        blocks = []
        r0 = 0
        for R in GROUPS:
            blocks.append(np.ascontiguousarray(base[:, :, :, r0:r0 + R]).ravel())
            r0 += R
        xp = np.concatenate(blocks)
        assert xp.shape[0] == X_TOTAL
        in_maps.append({"xp": xp, "ctp": ctp, "bias3": bias3, "ones3": ones3})
    return in_maps


def kernel(y_pred: np.ndarray, mask: np.ndarray, centers: np.ndarray,
           **run_kwargs) -> np.ndarray:
    in_maps = prep_inputs(y_pred, mask, centers)
    nc = get_nc()
    res = run_bass_kernel_spmd(nc, in_maps, core_ids=list(range(N_CORES)),
                               **run_kwargs)
    _CACHE["last_results"] = res
    out = np.concatenate(
        [r["out"].reshape(B_PER_CORE, VALID_T, K) for r in res.results], axis=0
    )
    return out.astype(np.float32, copy=False)


# revision 19
# speedup vs baseline: 1.3300x; 1.0105x over previous
"""Batch-assign-probability (VQ codebook softmax) kernel for 8 Trainium2 cores.

Math: for each valid row x (D=512), over K=256 centers c_k:
    softmax_k(-||x - c_k||^2) == softmax_k(2 x.c_k - ||c_k||^2)
(the ||x||^2 term is constant over k and cancels in softmax).

Sharding: batch B=16 split across 8 cores (2 batches = 2048 valid rows per
core); the small centers table is replicated. Host prep: slice the valid
(unmasked) timesteps, transpose x to [D, rows] so the contraction dim lands
on SBUF partitions, fold the 2x scale into ct = (2*centers)^T, and split
x / ct into bf16 hi+lo pairs for a 3-pass full-rate matmul:
    x.ct ~= xh.cth + xh.ctl + xl.cth        (error ~2e-4 relative)
The -||c||^2 bias is folded in as one contraction-dim-3 matmul against a
3-level bf16 split of the bias (ones rows on the x side). Host packs each
DMA's source region fully contiguous (8KB-per-partition runs).

Device (per core, Tile framework):
  - load ct hi/lo + bias once; stream x hi/lo in row-groups (small first
    group so the PE starts early, small last group so the exposed softmax
    tail is short); per 128-row tile: 12 bf16 matmuls + 1 bias matmul ->
    PSUM logits [128,256]; reduce_max(negate) -> ACT exp(bias=-max,
    accum sum) -> reciprocal -> scale -> group out DMA.
"""

import numpy as np
import ml_dtypes

import concourse.bacc as bacc
import concourse.tile as tile
from concourse import mybir
from concourse.bass_utils import run_bass_kernel_spmd

B, T, W, C, K = 16, 2048, 512, 1, 256
VALID_T = 1024
D = W * C                       # 512
N_CORES = 8
B_PER_CORE = B // N_CORES       # 2
ROWS = B_PER_CORE * VALID_T     # 2048 rows per core
P = 128
D_CHUNKS = D // P               # 4
GROUPS = [128, 384, 512, 512, 384, 128]   # rows per x/out DMA group
N_WARM_MM = 20                  # dummy matmuls to lift the PE HAM clock-gate
assert sum(GROUPS) == ROWS
X_TOTAL = P * 2 * D_CHUNKS * ROWS    # flat bf16 element count of x param

BF16_NP = ml_dtypes.bfloat16

_CACHE: dict = {}


def _build_bass():
    f32 = mybir.dt.float32
    bf16 = mybir.dt.bfloat16
    nc = bacc.Bacc()
    # x hi/lo, group-major, fully contiguous per group: for each group g
    # (R rows), block [128p, 2h, 4c, R] flattened.
    xp = nc.declare_dram_parameter("xp", [X_TOTAL], bf16, isOutput=False)
    # ct hi block then lo block, each [128p, 4c, 256k] contiguous.
    ctp = nc.declare_dram_parameter("ctp", [2 * P * D_CHUNKS * K], bf16,
                                    isOutput=False)
    bias3 = nc.declare_dram_parameter("bias3", [3, K], bf16, isOutput=False)
    ones3 = nc.declare_dram_parameter("ones3", [3, P], bf16, isOutput=False)
    out = nc.declare_dram_parameter("out", [ROWS, K], f32, isOutput=True)

    out_v = out.rearrange("(t p) k -> p t k", p=P)       # [128, 16, 256]
    ct_half = P * D_CHUNKS * K

    with tile.TileContext(nc) as tc:
        with (
            tc.tile_pool(name="singles", bufs=1) as singles,
            tc.tile_pool(name="xpool", bufs=1) as xpool,
            tc.tile_pool(name="opool", bufs=3) as opool,
            tc.tile_pool(name="small", bufs=8) as small,
            tc.tile_pool(name="psum", bufs=7, space="PSUM") as psum,
            tc.tile_pool(name="psum_warm", bufs=1, space="PSUM") as psum_warm,
        ):
            ct_sb = singles.tile([P, 2, D_CHUNKS, K], bf16)
            for h in range(2):
                nc.sync.dma_start(
                    out=ct_sb[:, h],
                    in_=ctp[h * ct_half:(h + 1) * ct_half].rearrange(
                        "(p c k) -> p c k", p=P, c=D_CHUNKS),
                )

            # x group DMAs up front — everything stays resident in SBUF.
            # bias/ones are slotted right after group 0 so the first tile's
            # bias matmul isn't gated on the later x dispatches.
            xgs = []
            xoff = 0
            bias_sb = ones_sb = None
            for g, R in enumerate(GROUPS):
                xg = xpool.tile([P, 2, D_CHUNKS, R], bf16, tag=f"xg{g}")
                n = P * 2 * D_CHUNKS * R
                nc.sync.dma_start(
                    out=xg[:],
                    in_=xp[xoff:xoff + n].rearrange(
                        "(p h c r) -> p h c r", p=P, h=2, c=D_CHUNKS),
                )
                xgs.append(xg)
                xoff += n
                if g == 0:
                    bias_sb = singles.tile([3, K], bf16)
                    nc.sync.dma_start(out=bias_sb[:], in_=bias3[:])
                    ones_sb = singles.tile([3, P], bf16)
                    nc.sync.dma_start(out=ones_sb[:], in_=ones3[:])

            # PE warm-up: dummy matmuls on scratch data keep the PE busy
            # through the HAM activity window while the first x DMA lands,
            # so the real matmul stream runs at 2.4 GHz from the start.
            warm_sb = singles.tile([P, P], bf16)
            nc.vector.memset(warm_sb[:], 0.0)
            warm_ps = psum_warm.tile([P, 64], f32, tag="warm")
            for _ in range(N_WARM_MM):
                nc.tensor.matmul(
                    warm_ps[:], lhsT=warm_sb[:], rhs=warm_sb[:, :64],
                    start=True, stop=True,
                )

            t0 = 0  # running 128-row tile index
            for g, R in enumerate(GROUPS):
                xg = xgs[g]
                subtiles = R // P
                og = opool.tile([P, subtiles, K], f32, tag="og")
                for s in range(subtiles):
                    rsl = slice(s * P, (s + 1) * P)
                    ps = psum.tile([P, K], f32)
                    first = True
                    for c in range(D_CHUNKS):
                        for xh_i, ct_i in ((0, 0), (0, 1), (1, 0)):
                            nc.tensor.matmul(
                                ps[:],
                                lhsT=xg[:, xh_i, c, rsl],
                                rhs=ct_sb[:, ct_i, c, :],
                                start=first,
                                stop=False,
                            )
                            first = False
                    nc.tensor.matmul(
                        ps[:],
                        lhsT=ones_sb[:, :],
                        rhs=bias_sb[:, :],
                        start=False,
                        stop=True,
                    )
                    negm = small.tile([P, 1], f32)
                    nc.vector.reduce_max(
                        out=negm[:], in_=ps[:], axis=mybir.AxisListType.X, negate=True
                    )
                    esum = small.tile([P, 1], f32)
                    nc.scalar.activation(
                        out=og[:, s, :],
                        in_=ps[:],
                        func=mybir.ActivationFunctionType.Exp,
                        bias=negm[:],
                        scale=1.0,
                        accum_out=esum[:],
                    )
                    rinv = small.tile([P, 1], f32)
                    nc.vector.reciprocal(out=rinv[:], in_=esum[:])
                    nc.vector.tensor_scalar_mul(og[:, s, :], og[:, s, :], rinv[:])
                nc.sync.dma_start(out=out_v[:, t0:t0 + subtiles, :], in_=og[:])
                t0 += subtiles
    nc.finalize()
    return nc


def get_nc():
    if "nc" not in _CACHE:
        _CACHE["nc"] = _build_bass()
    return _CACHE["nc"]


def _split_hi_lo(a: np.ndarray) -> tuple[np.ndarray, np.ndarray]:
    hi = a.astype(BF16_NP)
    lo = (a - hi.astype(np.float32)).astype(BF16_NP)
    return hi, lo


def prep_inputs(y_pred: np.ndarray, mask: np.ndarray, centers: np.ndarray):
    """Host-side shard prep: valid-timestep slice, per-core transpose,
    bf16 hi/lo splits, contiguous per-DMA packing."""
    x = np.ascontiguousarray(y_pred.reshape(B, T, D))
    masktime = np.asarray(mask).reshape(B, T, D)[0, :, 0]
    valid_idx = np.nonzero(masktime == 0)[0][:VALID_T]
    assert valid_idx.shape[0] == VALID_T
    if valid_idx[0] == 0 and valid_idx[-1] == VALID_T - 1:
        xv = x[:, :VALID_T]                    # [B, VALID_T, D]
    else:
        xv = x[:, valid_idx]

    centers = np.asarray(centers, dtype=np.float32)
    cth, ctl = _split_hi_lo((2.0 * centers).T)              # [D, K] each
    # [h, c, p, k] -> [h, p, c, k] contiguous
    ct_blocks = [
        np.ascontiguousarray(h.reshape(D_CHUNKS, P, K).transpose(1, 0, 2)).ravel()
        for h in (cth, ctl)
    ]
    ctp = np.ascontiguousarray(np.concatenate(ct_blocks))

    negc2 = -(centers.astype(np.float64) ** 2).sum(axis=1)  # [K]
    b1 = negc2.astype(BF16_NP)
    r1 = negc2 - b1.astype(np.float64)
    b2 = r1.astype(BF16_NP)
    b3 = (r1 - b2.astype(np.float64)).astype(BF16_NP)
    bias3 = np.ascontiguousarray(np.stack([b1, b2, b3]))    # [3, K]
    ones3 = np.ones((3, P), dtype=BF16_NP)

    in_maps = []
    for core in range(N_CORES):
        xc = xv[core * B_PER_CORE:(core + 1) * B_PER_CORE].reshape(ROWS, D)
        xTc = np.ascontiguousarray(xc.T)                    # [D, ROWS]
        xh, xl = _split_hi_lo(xTc)
        # [h, c, p, row] -> [p, h, c, row]
        base = np.stack([xh, xl]).reshape(2, D_CHUNKS, P, ROWS).transpose(2, 0, 1, 3)
        blocks = []
        r0 = 0
        for R in GROUPS:
            blocks.append(np.ascontiguousarray(base[:, :, :, r0:r0 + R]).ravel())
            r0 += R
        xp = np.concatenate(blocks)
        assert xp.shape[0] == X_TOTAL
        in_maps.append({"xp": xp, "ctp": ctp, "bias3": bias3, "ones3": ones3})
    return in_maps


def kernel(y_pred: np.ndarray, mask: np.ndarray, centers: np.ndarray,
           **run_kwargs) -> np.ndarray:
    in_maps = prep_inputs(y_pred, mask, centers)
    nc = get_nc()
    res = run_bass_kernel_spmd(nc, in_maps, core_ids=list(range(N_CORES)),
                               **run_kwargs)
    _CACHE["last_results"] = res
    out = np.concatenate(
        [r["out"].reshape(B_PER_CORE, VALID_T, K) for r in res.results], axis=0
    )
    return out.astype(np.float32, copy=False)
